# revision 109
# baseline (speedup 1.0000x reference)
"""Trainium2 Bass kernel for the EnhancedGNNDetector (3x GCN + GAT + pool + MLP).

Strategy (8 NeuronCores, SPMD single program):
  - Nodes sharded contiguously: core c owns dsts [c*6250, (c+1)*6250).
  - Edges partitioned by dst owner, sorted by dst, packed into 128-edge
    chunks per (4-block group, src-half) region; a chunk may span several
    dst blocks, each (chunk, block) pair is a "use" with its own one-hot
    column set.  Region chunk counts are padded to the cross-core max so one
    static program serves all cores.  Self loops are NOT materialized: their
    contribution is an identity matmul of the local table rows.
  - Per layer: node features are dinv-scaled, cast fp16, written to a local
    DRAM table, AllGathered to a full [50000, D] table; dma_gather pulls
    edge source rows; a one-hot S matrix per use (built on DVE from
    pair-duplicated dstrel values so every AP keeps a packed innermost dim
    -> 2x DVE mode) turns the segment-sum into PE matmuls in PSUM.
  - src index is int16 for dma_gather, so the full table is gathered in two
    halves (rows < 32768 and >= 32768).
  - Evicts are emitted stage-major across each group's 4 blocks (all Act
    scalings, then all PE transposes into one multi-region PSUM bank, then
    matmuls 2-deep) so independent blocks pipeline instead of ping-ponging
    engines serially.
  - GAT: table rows are [hg fp16 (256) | al_s f32-bitcast (8 slots) | pad]
    = 384 fp16 slots (768B).  The Wg matmul is extended with 8 extra columns
    [Was | Wad] (host-precomputed Wg contracted with a_src/a_dst) so al_s
    and al_d fall out of the same matmul.  Per-edge al_d needs no gather:
    each use's S column block is transposed on PE (A = S^T) and a tiny
    A @ ald_block matmul accumulates per-slot al_d in PSUM.  leaky_relu is
    computed on DVE as max(u, 0.2u) to avoid activation-table thrash.
    Softmax uses a global per-head shift applied after leaky_relu
    (softmax-invariant); den is computed by appending the per-head exp
    weights as extra message columns.  The exp weights are stored duplicated
    in pairs so the hg-weighting multiply also runs in 2x DVE mode.
  - NOTE: interleaved PSUM accumulation groups within one PSUM tile lose
    updates (ppool0/ppool1 must be separate tiles).
"""

import numpy as np
import concourse.bacc as bacc
import concourse.bass as bass
import concourse.mybir as mybir
import concourse.tile as tile
from concourse.bass_utils import run_bass_kernel_spmd

F16 = np.float16
N = 50000
E = 800000
NCORES = 8
NPC = N // NCORES            # 6250 nodes per core
NB = (NPC + 127) // 128      # 49 dst blocks per core
LASTB = NPC - 128 * (NB - 1)  # 106 rows in last block
HALF = 32768                 # int16 gather split
D_IN = 128
HID = 256
H3D = 128                    # dim of the L3/GAT aggregation tables
HEADS = 4
FH = 64
GSLOT = 384                  # GAT table row slots (fp16): 256 hg + 8 al_s(f32) + pad
OUT = 8
NEG = 0.2
GBLK = 4                     # blocks per gather group for GCN layers
GAT_SPLIT = 2                # GAT gathers per half-span (split 4-block span in 2)

fp16 = mybir.dt.float16
fp32 = mybir.dt.float32
i16 = mybir.dt.int16
ALU = mybir.AluOpType
ACT = mybir.ActivationFunctionType


# --------------------------------------------------------------------------
# host-side schedule + per-core streams
# --------------------------------------------------------------------------

def _preprocess(x, edge_index):
    # Self loops are NOT materialized as edges: their contribution is added
    # on-device via identity matmuls against the local table rows.  Edges are
    # packed across the blocks of each (group, half) region so padding is one
    # chunk-rounding per region (plus cross-core max) instead of per block.
    src = edge_index[0].astype(np.int64)
    dst = edge_index[1].astype(np.int64)
    deg = np.bincount(dst, minlength=N).astype(np.float32) + 1.0  # + self loop
    dinv = (1.0 / np.sqrt(deg)).astype(np.float32)

    order = np.argsort(dst, kind="stable")
    s_src, s_dst = src[order], dst[order]

    core = s_dst // NPC
    blk = (s_dst % NPC) // 128
    half = (s_src >= HALF).astype(np.int64)

    # edge lists per (core, block, half)
    key = (core * NB + blk) * 2 + half
    korder = np.argsort(key, kind="stable")   # stable: keeps dst order inside
    k_src, k_dst, k_key = s_src[korder], s_dst[korder], key[korder]
    bounds = np.searchsorted(k_key, np.arange(NCORES * NB * 2 + 1))
    cnt = (bounds[1:] - bounds[:-1]).reshape(NCORES, NB, 2)

    groups = [tuple(range(g, min(g + GBLK, NB))) for g in range(0, NB, GBLK)]

    # packed chunk counts per (group, half), cross-core max
    layout = []    # (lo_s, lo_n, hi_s, hi_n, g, lo_us, lo_un, hi_us, hi_un)
    use_flat = []  # canonical use list: (k_chunk_global, b)
    total_uses = np.zeros(NB, np.int64)
    cpos = 0
    upos = 0
    for g in groups:
        entry = []
        for h in (0, 1):
            tot = cnt[:, list(g), h].sum(axis=1)          # [NCORES]
            n_ch = int((-(-tot // 128)).max())
            # per-block chunk windows across cores
            ustart = upos
            ulist = []
            kmin, kmax = {}, {}
            for b_i, b in enumerate(g):
                starts = cnt[:, list(g)[:b_i], h].sum(axis=1)
                ends = starts + cnt[:, b, h]
                if (ends == starts).all():
                    continue
                kmin[b] = int(np.floor(starts.min() / 128))
                kmax[b] = min(int(-(-ends.max() // 128)), n_ch)
            for k in range(n_ch):
                for b in g:
                    if b in kmin and kmin[b] <= k < kmax[b]:
                        ulist.append((cpos + k, b))
                        total_uses[b] += 1
            use_flat.extend(ulist)
            upos += len(ulist)
            entry.append((cpos, n_ch, ustart, len(ulist)))
            cpos += n_ch
        (lo_s, lo_n, lo_us, lo_un), (hi_s, hi_n, hi_us, hi_un) = entry
        layout.append((lo_s, lo_n, hi_s, hi_n, g, lo_us, lo_un, hi_us, hi_un))
    NCH = cpos
    NUSE = upos

    # per-core streams
    def wrap(stream):
        return np.ascontiguousarray(np.tile(stream.reshape(-1, 16).T.copy(), (8, 1)))

    idxs_all, idxd_all, dstrel_all = [], [], []
    for c in range(NCORES):
        idx_stream = np.zeros(NCH * 128, np.int16)
        idxd_stream = np.zeros(NCH * 128, np.int16)
        rel_stream = np.full(NUSE * 128, -1.0, np.float32)
        for (lo_s, lo_n, hi_s, hi_n, g, lo_us, lo_un, hi_us, hi_un) in layout:
            for h, cstart, ustart, un in ((0, lo_s, lo_us, lo_un),
                                          (1, hi_s, hi_us, hi_un)):
                base = cstart * 128
                p = base
                bpos = {}
                for b in g:
                    kk = (c * NB + b) * 2 + h
                    e0, e1 = bounds[kk], bounds[kk + 1]
                    n = e1 - e0
                    idx_stream[p:p + n] = (k_src[e0:e1] - (HALF if h else 0)
                                           ).astype(np.int16)
                    idxd_stream[p:p + n] = (k_dst[e0:e1] - c * NPC).astype(np.int16)
                    bpos[b] = (p - base, n, e0)
                    p += n
                for uu in range(un):
                    kg, b = use_flat[ustart + uu]
                    k = kg - cstart
                    s0, n, e0 = bpos.get(b, (0, 0, 0))
                    a = max(s0, k * 128)
                    z = min(s0 + n, (k + 1) * 128)
                    if a < z:
                        rel_stream[(ustart + uu) * 128 + (a - k * 128):
                                   (ustart + uu) * 128 + (z - k * 128)] = (
                            k_dst[e0 + a - s0:e0 + z - s0] - c * NPC - b * 128
                        ).astype(np.float32)
        idxs_all.append(wrap(idx_stream))
        idxd_all.append(wrap(idxd_stream))
        dstrel_all.append(rel_stream)

    dinv_blocks = []
    for c in range(NCORES):
        dv = np.ones(NB * 128, np.float32)
        dv[:NPC] = dinv[c * NPC:(c + 1) * NPC]
        dinv_blocks.append(np.ascontiguousarray(dv.reshape(NB, 128).T))  # [128, NB]

    return {
        "layout": layout, "use_flat": use_flat, "NCH": NCH, "NUSE": NUSE,
        "total_uses": total_uses,
        "idxs": idxs_all, "idxd": idxd_all, "dstrel": dstrel_all, "dinv": dinv_blocks,
    }


# --------------------------------------------------------------------------
# device program
# --------------------------------------------------------------------------

def _build(sched, repeat=1, no_cc=False):
    NCH = sched["NCH"]
    NUSE = sched["NUSE"]
    layout = sched["layout"]
    use_flat = sched["use_flat"]
    total_uses = sched["total_uses"]
    MAXSPC = max(max(e[1], e[3]) for e in layout)   # max chunks per span

    nc = bacc.Bacc("TRN2", target_bir_lowering=False, debug=False,
                   num_devices=NCORES, num_swdge_queues=4)

    # ---------------- external tensors ----------------
    xs = nc.dram_tensor("xs", [NPC, D_IN], fp32, kind="ExternalInput")
    idxs_d = nc.dram_tensor("idxs_d", [128, NCH * 8], i16, kind="ExternalInput")
    dstrel_d = nc.dram_tensor("dstrel_d", [128, NUSE * 2], fp16, kind="ExternalInput")
    dinv_d = nc.dram_tensor("dinv_d", [128, NB], fp32, kind="ExternalInput")
    w1_d = nc.dram_tensor("w1_d", [128, HID], fp16, kind="ExternalInput")
    w2_d = nc.dram_tensor("w2_d", [128, 2 * HID], fp16, kind="ExternalInput")
    w3_d = nc.dram_tensor("w3_d", [128, 2 * H3D], fp16, kind="ExternalInput")
    wg_d = nc.dram_tensor("wg_d", [128, HID + 8], fp16, kind="ExternalInput")
    b1_d = nc.dram_tensor("b1_d", [1, HID], fp16, kind="ExternalInput")
    b2_d = nc.dram_tensor("b2_d", [1, HID], fp16, kind="ExternalInput")
    b3_d = nc.dram_tensor("b3_d", [128, H3D], fp32, kind="ExternalInput")
    bg_d = nc.dram_tensor("bg_d", [128, HID], fp32, kind="ExternalInput")
    wc1_d = nc.dram_tensor("wc1_d", [128, 2 * 128], fp32, kind="ExternalInput")
    wc2_d = nc.dram_tensor("wc2_d", [128, 64], fp32, kind="ExternalInput")
    wc3_d = nc.dram_tensor("wc3_d", [64, 8], fp32, kind="ExternalInput")
    bc1_d = nc.dram_tensor("bc1_d", [128, 1], fp32, kind="ExternalInput")
    bc2_d = nc.dram_tensor("bc2_d", [64, 1], fp32, kind="ExternalInput")
    bc3_d = nc.dram_tensor("bc3_d", [8, 1], fp32, kind="ExternalInput")
    rowmask_d = nc.dram_tensor("rowmask_d", [128, 1], fp32, kind="ExternalInput")
    out_d = nc.dram_tensor("out_d", [8, 1], fp32, kind="ExternalOutput")

    # internal DRAM tables
    g1loc = nc.dram_tensor("g1loc", [NPC, D_IN], fp16)
    g1full = nc.dram_tensor("g1full", [N, D_IN], fp16, addr_space="Shared")
    g2loc = nc.dram_tensor("g2loc", [NPC, HID], fp16)
    g2full = nc.dram_tensor("g2full", [N, HID], fp16, addr_space="Shared")
    g3loc = nc.dram_tensor("g3loc", [NPC, H3D], fp16)
    g3full = nc.dram_tensor("g3full", [N, H3D], fp16, addr_space="Shared")
    gtloc = nc.dram_tensor("gtloc", [NPC, GSLOT], fp16)
    gtfull = nc.dram_tensor("gtfull", [N, GSLOT], fp16, addr_space="Shared")
    arin = nc.dram_tensor("arin", [128, 2], fp32)
    arout = nc.dram_tensor("arout", [128, 2], fp32, addr_space="Shared")

    RG = [list(range(NCORES))]

    with tile.TileContext(nc) as tc:
        import contextlib
        es = contextlib.ExitStack()
        with es:
            pers = es.enter_context(tc.tile_pool(name="pers", bufs=1))
            # ---------- persistent SBUF ----------
            idxs = pers.tile([128, NCH * 8], i16)
            nc.sync.dma_start(idxs[:], idxs_d[:])
            dstrel = pers.tile([128, NUSE, 2], fp16)
            nc.sync.dma_start(dstrel[:].rearrange("p k b -> p (k b)"), dstrel_d[:])
            dinv = pers.tile([128, NB], fp32)
            nc.sync.dma_start(dinv[:], dinv_d[:])

            w1 = pers.tile([128, HID], fp16); nc.sync.dma_start(w1[:], w1_d[:])
            w2 = pers.tile([128, 2 * HID], fp16); nc.sync.dma_start(w2[:], w2_d[:])
            w3 = pers.tile([128, 2 * H3D], fp16); nc.sync.dma_start(w3[:], w3_d[:])
            wg = pers.tile([128, HID + 8], fp16); nc.sync.dma_start(wg[:], wg_d[:])
            b1r = pers.tile([1, HID], fp16); nc.sync.dma_start(b1r[:], b1_d[:])
            b2r = pers.tile([1, HID], fp16); nc.sync.dma_start(b2r[:], b2_d[:])
            b3r = pers.tile([128, H3D], fp32); nc.sync.dma_start(b3r[:], b3_d[:])
            bgr = pers.tile([128, HID], fp32); nc.sync.dma_start(bgr[:], bg_d[:])
            wc1 = pers.tile([128, 2 * 128], fp32); nc.sync.dma_start(wc1[:], wc1_d[:])
            wc2 = pers.tile([128, 64], fp32); nc.sync.dma_start(wc2[:], wc2_d[:])
            wc3 = pers.tile([64, 8], fp32); nc.sync.dma_start(wc3[:], wc3_d[:])
            bc1 = pers.tile([128, 1], fp32); nc.sync.dma_start(bc1[:], bc1_d[:])
            bc2 = pers.tile([64, 1], fp32); nc.sync.dma_start(bc2[:], bc2_d[:])
            bc3 = pers.tile([8, 1], fp32); nc.sync.dma_start(bc3[:], bc3_d[:])
            rowmask = pers.tile([128, 1], fp32); nc.sync.dma_start(rowmask[:], rowmask_d[:])

            iota_i = pers.tile([128, 128], i16)
            nc.gpsimd.iota(iota_i[:], pattern=[[1, 128]], base=0, channel_multiplier=0)
            iota_f = pers.tile([128, 128], fp16)
            nc.vector.tensor_copy(iota_f[:], iota_i[:])
            iop_i = pers.tile([128, 1], i16)
            nc.gpsimd.iota(iop_i[:], pattern=[[1, 1]], base=0, channel_multiplier=1)
            iop_f = pers.tile([128, 1], fp16)
            nc.vector.tensor_copy(iop_f[:], iop_i[:])
            ident = pers.tile([128, 128], fp16)
            nc.vector.tensor_tensor(
                ident[:], iop_f[:].broadcast_to([128, 128]), iota_f[:],
                op=ALU.is_equal)
            ones_r = pers.tile([1, 128], fp16)
            nc.vector.memset(ones_r[:], 1.0)
            ones_c = pers.tile([128, 1], fp16)
            nc.vector.memset(ones_c[:], 1.0)

            als_all = pers.tile([128, NB, HEADS], fp32)
            ald_all = pers.tile([128, NB, HEADS], fp32)
            crep = pers.tile([128, HEADS], fp32)

            def rows(b):
                return LASTB if b == NB - 1 else 128

            # ---------- helpers ----------
            def transpose_to_sbuf(pool, psum_pool, src16, nslab, tag):
                """src16 [128, nslab*128] fp16 -> returns [128, nslab, 128] fp16."""
                out = pool.tile([128, nslab, 128], fp16, tag=tag, name=f"tT_{tag}")
                for s in range(nslab):
                    pt = psum_pool.tile([128, 128], fp16, tag="tr", name="pt_tr", bufs=1)
                    nc.tensor.transpose(pt[:], src16[:, s * 128:(s + 1) * 128], ident[:])
                    nc.scalar.copy(out[:, s, :], pt[:])
                return out

            qctr = [0]

            def next_q():
                qctr[0] += 1
                return qctr[0] % 4

            def gather_into(m_tile, table, start_chunk, n_chunks, elem):
                nc.gpsimd.dma_gather(
                    m_tile[:, 0:n_chunks, :], table,
                    idxs[:, start_chunk * 8:(start_chunk + n_chunks) * 8],
                    num_idxs=n_chunks * 128, num_idxs_reg=n_chunks * 128,
                    elem_size=elem, single_packet=False, queue_num=next_q())

            def build_S(pool, k0, n_chunks, tag, bufs=None):
                # pair-duplicated dstrel keeps every AP's innermost dim packed
                # (stride 1, count 2) so DVE runs in 2x mode.
                S = pool.tile([128, n_chunks, 128], fp16, tag=tag, name=f"S_{tag}",
                              bufs=bufs)
                nc.vector.tensor_tensor(
                    S[:].rearrange("p k (a b) -> p k a b", b=2),
                    dstrel[:, k0:k0 + n_chunks, :].unsqueeze(2).broadcast_to(
                        [128, n_chunks, 64, 2]),
                    iota_f[:].rearrange("p (a b) -> p a b", b=2).unsqueeze(1).broadcast_to(
                        [128, n_chunks, 64, 2]),
                    op=ALU.is_equal)
                return S

            def maybe_cc(kind, op, replica_groups, ins, outs):
                if no_cc:
                    nc.sync.dma_start(outs[0].tensor[0:ins[0].shape[0]], ins[0])
                else:
                    nc.gpsimd.collective_compute(kind, op, replica_groups=replica_groups,
                                                 ins=ins, outs=outs)

            def run_body(rep):
                # ================= phase 0: g1 = dinv * x =================
                with tc.tile_pool(name=f"p0_{rep}", bufs=3) as p0:
                    for g0 in range(0, NB - 1, 4):
                        gn = min(4, NB - 1 - g0)
                        xt = p0.tile([128, 4, D_IN], fp32, name="xt")
                        nc.sync.dma_start(
                            xt[:, 0:gn, :],
                            xs[g0 * 128:(g0 + gn) * 128, :].rearrange(
                                "(g p) d -> p g d", p=128))
                        gt = p0.tile([128, 4, D_IN], fp16, name="gt")
                        nc.vector.tensor_tensor(
                            gt[:, 0:gn, :], xt[:, 0:gn, :],
                            dinv[:, g0:g0 + gn].unsqueeze(2).broadcast_to(
                                [128, gn, D_IN]),
                            op=ALU.mult)
                        nc.sync.dma_start(
                            g1loc[g0 * 128:(g0 + gn) * 128, :].rearrange(
                                "(g p) d -> p g d", p=128),
                            gt[:, 0:gn, :])
                    b = NB - 1
                    r = rows(b)
                    xtl = p0.tile([128, D_IN], fp32, name="xtl")
                    nc.sync.dma_start(xtl[:r, :], xs[b * 128:b * 128 + r, :])
                    gtl = p0.tile([128, D_IN], fp16, name="gtl")
                    nc.vector.tensor_scalar(gtl[:], xtl[:], dinv[:, b:b + 1], None, op0=ALU.mult)
                    nc.sync.dma_start(g1loc[b * 128:b * 128 + r, :], gtl[:r, :])

                maybe_cc("AllGather", ALU.bypass, RG, [g1loc[:]], [g1full[:]])

                # ================= GCN layer runner =================
                h1_pool = tc.tile_pool(name=f"h1pool_{rep}", bufs=1)
                h1_ctx = h1_pool.__enter__()
                h1_all = h1_ctx.tile([128, NB, HID], fp32)

                def gcn_layer(lname, table_full, table_loc, D, evict_fn):
                    """Aggregate A @ g (table rows = D fp16) into per-block psum,
                    then call evict_fn(b, psum_ap, pools...) per block.  The
                    self-loop term is an identity matmul of the local rows."""
                    with (tc.tile_pool(name=f"{lname}_sb_{rep}", bufs=2) as lp,
                          tc.tile_pool(name=f"{lname}_ps_{rep}", bufs=5, space="PSUM") as pp,
                          tc.tile_pool(name=f"{lname}_wps_{rep}", bufs=2, space="PSUM") as wp):
                        tab_lo = table_full[0:HALF, :]
                        tab_hi = table_full[HALF:N, :]
                        for (lo_s, lo_n, hi_s, hi_n, g, lo_us, lo_un,
                             hi_us, hi_un) in layout:
                            gn = len(g)
                            gl = lp.tile([128, GBLK, D], fp16, tag="gl", name="gl")
                            g0 = g[0]
                            if g[-1] == NB - 1:
                                nc.vector.memset(gl[:], 0.0)
                                full = gn - 1
                                if full:
                                    nc.sync.dma_start(
                                        gl[:, 0:full, :],
                                        table_loc[g0 * 128:(g0 + full) * 128, :]
                                        .rearrange("(g p) d -> p g d", p=128))
                                nc.sync.dma_start(
                                    gl[:LASTB, full, :],
                                    table_loc[(NB - 1) * 128:NPC, :])
                            else:
                                nc.sync.dma_start(
                                    gl[:, 0:gn, :],
                                    table_loc[g0 * 128:(g0 + gn) * 128, :]
                                    .rearrange("(g p) d -> p g d", p=128))
                            paggs = {}
                            done = {}
                            for b_i, b in enumerate(g):
                                paggs[b] = pp.tile([128, D], fp32, tag="agg", name="pagg")
                                done[b] = 0
                                nc.tensor.matmul(
                                    paggs[b][:], ident[:], gl[:, b_i, :],
                                    start=True, stop=(int(total_uses[b]) == 0))
                            for (cstart, n_ch, us, un, tab) in (
                                    (lo_s, lo_n, lo_us, lo_un, tab_lo),
                                    (hi_s, hi_n, hi_us, hi_un, tab_hi)):
                                if n_ch == 0:
                                    continue
                                m = lp.tile([128, n_ch, D], fp16, tag="m", name="m", bufs=3)
                                gather_into(m, tab, cstart, n_ch, D)
                                S = build_S(lp, us, un, "s")
                                for uu in range(un):
                                    kg, b = use_flat[us + uu]
                                    done[b] += 1
                                    nc.tensor.matmul(
                                        paggs[b][:], S[:, uu, :], m[:, kg - cstart, :],
                                        start=False, stop=(done[b] == int(total_uses[b])))
                            evict_fn(g, paggs, lp, wp)

                def pipe2(items, fA, fB):
                    # A0 A1 B0 A2 B1 ... : keeps 2 PSUM tiles live, overlaps
                    prev = None
                    for it in items:
                        fA(it)
                        if prev is not None:
                            fB(prev)
                        prev = it
                    if prev is not None:
                        fB(prev)

                def write_group(tab, tile_, g, D):
                    # one batched DRAM write per group instead of per block
                    g0, gn = g[0], len(g)
                    if g[-1] == NB - 1:
                        full = gn - 1
                        if full:
                            nc.sync.dma_start(
                                tab[g0 * 128:(g0 + full) * 128, :].rearrange(
                                    "(g p) d -> p g d", p=128),
                                tile_[:, 0:full, :])
                        nc.sync.dma_start(tab[(NB - 1) * 128:NPC, :],
                                          tile_[:LASTB, full, :])
                    else:
                        nc.sync.dma_start(
                            tab[g0 * 128:(g0 + gn) * 128, :].rearrange(
                                "(g p) d -> p g d", p=128),
                            tile_[:, 0:gn, :])

                # ---------- layer 1 (stage-major across the group) ----------
                def evict1(g, paggs, lp, wp):
                    a1s = {}
                    for b in g:
                        t = lp.tile([128, D_IN], fp16, tag="ev1", name="a1s", bufs=5)
                        nc.scalar.mul(t[:], paggs[b][:], dinv[:, b:b + 1])
                        a1s[b] = t
                    ptr = wp.tile([128, len(g), 128], fp16, tag="tr", name="ptr1", bufs=1)
                    for i, b in enumerate(g):
                        nc.tensor.transpose(ptr[:, i, :], a1s[b][:], ident[:])
                    a1T = {}
                    for i, b in enumerate(g):
                        t = lp.tile([128, 128], fp16, tag="ev1T", name="a1T", bufs=5)
                        nc.scalar.copy(t[:], ptr[:, i, :])
                        a1T[b] = t

                    phs = {}

                    def mm1(b):
                        ph = wp.tile([128, HID], fp32, tag="wout", name="ph1")
                        nc.tensor.matmul(ph[:], a1T[b][:], w1[:], start=True, stop=False)
                        nc.tensor.matmul(ph[:], ones_r[:], b1r[:], start=False, stop=True)
                        phs[b] = ph

                    g2g = lp.tile([128, GBLK, HID], fp16, tag="ev1g", name="g2g",
                                  bufs=2)

                    def out1(b):
                        h1t = h1_all[:, b, :]
                        nc.scalar.activation(h1t, phs[b][:], ACT.Relu)
                        nc.vector.tensor_scalar(g2g[:, g.index(b), :], h1t,
                                                dinv[:, b:b + 1], None, op0=ALU.mult)

                    pipe2(list(g), mm1, out1)
                    write_group(g2loc, g2g, g, HID)

                gcn_layer("L1", g1full, g1loc, D_IN, evict1)
                maybe_cc("AllGather", ALU.bypass, RG, [g2loc[:]], [g2full[:]])

                # ---------- layer 2 (+ residual + L3 transform) ----------
                def evict2(g, paggs, lp, wp):
                    a2s = {}
                    for b in g:
                        t = lp.tile([128, HID], fp16, tag="ev2", name="a2s", bufs=5)
                        nc.scalar.mul(t[:], paggs[b][:], dinv[:, b:b + 1])
                        a2s[b] = t
                    ptr = wp.tile([128, 2 * len(g), 128], fp16, tag="tr", name="ptr2", bufs=1)
                    for i, b in enumerate(g):
                        nc.tensor.transpose(ptr[:, 2 * i, :], a2s[b][:, 0:128], ident[:])
                        nc.tensor.transpose(ptr[:, 2 * i + 1, :], a2s[b][:, 128:256], ident[:])
                    a2T = {}
                    for i, b in enumerate(g):
                        t = lp.tile([128, 2, 128], fp16, tag="ev2T", name="a2T", bufs=5)
                        nc.scalar.copy(t[:, 0, :], ptr[:, 2 * i, :])
                        nc.scalar.copy(t[:, 1, :], ptr[:, 2 * i + 1, :])
                        a2T[b] = t

                    phs, h2t16s = {}, {}

                    def mm2(b):
                        ph = wp.tile([128, HID], fp32, tag="wout", name="ph2")
                        nc.tensor.matmul(ph[:], a2T[b][:, 0, :], w2[:, 0:HID], start=True, stop=False)
                        nc.tensor.matmul(ph[:], a2T[b][:, 1, :], w2[:, HID:2 * HID], start=False, stop=False)
                        nc.tensor.matmul(ph[:], ones_r[:], b2r[:], start=False, stop=True)
                        phs[b] = ph

                    def res2(b):
                        r2 = lp.tile([128, HID], fp32, tag="ev2r", name="r2", bufs=5)
                        nc.scalar.activation(r2[:], phs[b][:], ACT.Relu)
                        h2t16 = lp.tile([128, HID], fp16, tag="ev2h6", name="h2t16", bufs=5)
                        nc.vector.tensor_tensor(h2t16[:], r2[:], h1_all[:, b, :], op=ALU.add)
                        h2t16s[b] = h2t16

                    pipe2(list(g), mm2, res2)

                    ptr2 = wp.tile([128, 2 * len(g), 128], fp16, tag="tr", name="ptr2b", bufs=1)
                    for i, b in enumerate(g):
                        nc.tensor.transpose(ptr2[:, 2 * i, :], h2t16s[b][:, 0:128], ident[:])
                        nc.tensor.transpose(ptr2[:, 2 * i + 1, :], h2t16s[b][:, 128:256], ident[:])
                    h2T = {}
                    for i, b in enumerate(g):
                        t = lp.tile([128, 2, 128], fp16, tag="ev2hT", name="h2T", bufs=5)
                        nc.scalar.copy(t[:, 0, :], ptr2[:, 2 * i, :])
                        nc.scalar.copy(t[:, 1, :], ptr2[:, 2 * i + 1, :])
                        h2T[b] = t

                    pt3s = {}

                    def mm3(b):
                        pt3 = wp.tile([128, H3D], fp32, tag="wout", name="pt3")
                        nc.tensor.matmul(pt3[:], h2T[b][:, 0, :], w3[:, 0:H3D], start=True, stop=False)
                        nc.tensor.matmul(pt3[:], h2T[b][:, 1, :], w3[:, H3D:2 * H3D], start=False, stop=True)
                        pt3s[b] = pt3

                    g3g = lp.tile([128, GBLK, H3D], fp16, tag="ev2g", name="g3g",
                                  bufs=2)

                    def out3(b):
                        nc.scalar.mul(g3g[:, g.index(b), :], pt3s[b][:],
                                      dinv[:, b:b + 1])

                    pipe2(list(g), mm3, out3)
                    write_group(g3loc, g3g, g, H3D)

                gcn_layer("L2", g2full, g2loc, HID, evict2)
                h1_pool.__exit__(None, None, None)
                maybe_cc("AllGather", ALU.bypass, RG, [g3loc[:]], [g3full[:]])

                # ---------- layer 3 aggregation + GAT prep ----------
                def evict3(g, paggs, lp, wp):
                    h3t16s = {}
                    for b in g:
                        a3b = lp.tile([128, H3D], fp32, tag="ev3b", name="a3b", bufs=5)
                        nc.vector.scalar_tensor_tensor(
                            a3b[:], paggs[b][:], dinv[:, b:b + 1], b3r[:],
                            op0=ALU.mult, op1=ALU.add)
                        h3t16 = lp.tile([128, H3D], fp16, tag="ev3h", name="h3t16", bufs=5)
                        nc.scalar.activation(h3t16[:], a3b[:], ACT.Relu)
                        h3t16s[b] = h3t16
                    ptr = wp.tile([128, len(g), 128], fp16, tag="tr", name="ptr3", bufs=1)
                    for i, b in enumerate(g):
                        nc.tensor.transpose(ptr[:, i, :], h3t16s[b][:], ident[:])
                    h3T = {}
                    for i, b in enumerate(g):
                        t = lp.tile([128, 128], fp16, tag="ev3T", name="h3T", bufs=5)
                        nc.scalar.copy(t[:], ptr[:, i, :])
                        h3T[b] = t

                    phgs = {}

                    def mmg(b):
                        # wg is [Wg | Was | Wad]: one matmul yields hg AND the
                        # per-head al_s/al_d dot products (cols 256..263)
                        phg = wp.tile([128, HID + 8], fp32, tag="wout", name="phg")
                        nc.tensor.matmul(phg[:], h3T[b][:], wg[:], start=True, stop=True)
                        phgs[b] = phg

                    tabg = lp.tile([128, GBLK, GSLOT], fp16, tag="ev3tab",
                                   name="tabg", bufs=2)

                    def outg(b):
                        phg = phgs[b]
                        i = g.index(b)
                        nc.scalar.copy(als_all[:, b, :], phg[:, HID:HID + 4])
                        nc.scalar.copy(ald_all[:, b, :], phg[:, HID + 4:HID + 8])
                        # table row: [hg fp16 | al_s f32 | pad]
                        nc.scalar.copy(tabg[:, i, 0:HID], phg[:, 0:HID])
                        nc.scalar.copy(tabg[:, i, HID:HID + 8].bitcast(fp32),
                                       phg[:, HID:HID + 4])

                    pipe2(list(g), mmg, outg)
                    write_group(gtloc, tabg, g, GSLOT)

                gcn_layer("L3", g3full, g3loc, H3D, evict3)

                # shift constants c[h] = leaky(max al_s + max al_d)
                cps = contextlib.ExitStack()
                cp = cps.enter_context(tc.tile_pool(name=f"cp_{rep}", bufs=1))
                cpp = cps.enter_context(tc.tile_pool(name=f"cpp_{rep}", bufs=1, space="PSUM"))
                m1 = cp.tile([128, HEADS], fp32)
                nc.vector.tensor_reduce(
                    m1[:], als_all[:].rearrange("p b h -> p h b"),
                    axis=mybir.AxisListType.X, op=ALU.max)
                m2 = cp.tile([128, HEADS], fp32)
                nc.vector.tensor_reduce(
                    m2[:], ald_all[:].rearrange("p b h -> p h b"),
                    axis=mybir.AxisListType.X, op=ALU.max)
                m1_16 = cp.tile([128, HEADS], fp16)
                nc.vector.tensor_copy(m1_16[:], m1[:])
                m2_16 = cp.tile([128, HEADS], fp16)
                nc.vector.tensor_copy(m2_16[:], m2[:])
                pmt1 = cpp.tile([HEADS, 128], fp16, tag="pmt1", name="pmt1")
                nc.tensor.transpose(pmt1[:], m1_16[:], ident[:])
                pmt2 = cpp.tile([HEADS, 128], fp16, tag="pmt2", name="pmt2")
                nc.tensor.transpose(pmt2[:], m2_16[:], ident[:])
                mt = cp.tile([HEADS, 2 * 128], fp32)
                nc.scalar.copy(mt[:, 0:128], pmt1[:])
                nc.scalar.copy(mt[:, 128:256], pmt2[:])
                ms = cp.tile([HEADS, 2], fp32)
                nc.vector.tensor_reduce(
                    ms[:], mt[:].rearrange("p (a j) -> p a j", a=2),
                    axis=mybir.AxisListType.X, op=ALU.max)
                ub = cp.tile([HEADS, 1], fp32)
                nc.vector.tensor_tensor(ub[:], ms[:, 0:1], ms[:, 1:2], op=ALU.add)
                ub2 = cp.tile([HEADS, 1], fp32)
                nc.vector.tensor_scalar(ub2[:], ub[:], 0.2, None, op0=ALU.mult)
                cc = cp.tile([HEADS, 1], fp32)
                nc.vector.tensor_tensor(cc[:], ub[:], ub2[:], op=ALU.max)
                cc16 = cp.tile([HEADS, 1], fp16)
                nc.vector.tensor_copy(cc16[:], cc[:])
                pcr = cpp.tile([1, HEADS], fp16)
                nc.tensor.transpose(pcr[:], cc16[:HEADS, :], ident[0:HEADS, 0:HEADS])
                pcr_sb = cp.tile([1, HEADS], fp16)
                nc.scalar.copy(pcr_sb[:], pcr[:])
                pcrep = cpp.tile([128, HEADS], fp32)
                nc.tensor.matmul(pcrep[:], ones_r[:], pcr_sb[:], start=True, stop=True)
                nc.scalar.copy(crep[:], pcrep[:])
                cps.close()

                maybe_cc("AllGather", ALU.bypass, RG, [gtloc[:]], [gtfull[:]])

                # ================= GAT layer =================
                plp_cm = tc.tile_pool(name=f"pool_ps_{rep}", bufs=1, space="PSUM")
                plp = plp_cm.__enter__()
                ppool0 = plp.tile([128, 1], fp32, tag="pp0", name="ppool0")
                ppool1 = plp.tile([128, 1], fp32, tag="pp1", name="ppool1")
                with (tc.tile_pool(name=f"gat_sb_{rep}", bufs=2) as gp,
                      tc.tile_pool(name=f"gat_ps_{rep}", bufs=4, space="PSUM") as gpp):
                    tab_lo = gtfull[0:HALF, :]
                    tab_hi = gtfull[HALF:N, :]
                    DM = HID + 12       # message cols: 256 hg + (8 al_s slots) + 4 exp
                    done = {b: 0 for b in range(NB)}
                    paggs = {}
                    ald16 = gp.tile([128, NB, HEADS], fp16, tag="gald16",
                                    name="ald16", bufs=1)
                    nc.vector.tensor_copy(ald16[:], ald_all[:])

                    # self-loop attention weights for every local node, paired
                    wself = gp.tile([128, NB, HEADS, 2], fp16, tag="gws",
                                    name="wself", bufs=1)
                    us_t = gp.tile([128, NB, HEADS], fp32, tag="gus", name="us_t",
                                   bufs=1)
                    nc.vector.tensor_tensor(us_t[:], als_all[:], ald_all[:], op=ALU.add)
                    nc.vector.scalar_tensor_tensor(
                        us_t[:], us_t[:], NEG, us_t[:], op0=ALU.mult, op1=ALU.max)
                    nc.vector.tensor_tensor(
                        us_t[:], us_t[:],
                        crep[:].unsqueeze(1).broadcast_to([128, NB, HEADS]),
                        op=ALU.subtract)
                    nc.scalar.activation(wself[:, :, :, 0], us_t[:], ACT.Exp)
                    nc.scalar.activation(wself[:, :, :, 1], us_t[:], ACT.Exp)

                    def gat_span(start, n_ch, us, un, tab):
                        if n_ch == 0:
                            return
                        m = gp.tile([128, n_ch, GSLOT], fp16, tag="gm", name="gm", bufs=3)
                        gather_into(m, tab, start, n_ch, GSLOT)
                        S = build_S(gp, us, un, "gs", bufs=3)
                        # per-slot al_d without a DMA gather: transpose each
                        # S column block on PE (A = S^T), then a tiny matmul
                        # A^T @ ald_block accumulates al_d per edge slot.
                        # Stage-major in batches of 8 with one Act copy per
                        # batch keeps the chain off the critical path.
                        aldp = gpp.tile([128, MAXSPC, HEADS], fp32,
                                        tag="galdp", name="aldp", bufs=1)
                        uses_span = [use_flat[us + uu] for uu in range(un)]
                        firsts, lasts = {}, {}
                        for uu, (kg, b) in enumerate(uses_span):
                            k = kg - start
                            if k not in firsts:
                                firsts[k] = uu
                            lasts[k] = uu
                        for b0 in range(0, un, 8):
                            bl = min(8, un - b0)
                            trr = gpp.tile([128, 8, 128], fp16, tag="gatr",
                                           name="trr", bufs=1)
                            Abuf = gp.tile([128, 8, 128], fp16, tag="gA",
                                           name="gA", bufs=3)
                            for j in range(bl):
                                nc.tensor.transpose(trr[:, j, :], S[:, b0 + j, :],
                                                    ident[:])
                            nc.scalar.copy(Abuf[:, 0:bl, :], trr[:, 0:bl, :])
                            for j in range(bl):
                                kg, b = uses_span[b0 + j]
                                k = kg - start
                                nc.tensor.matmul(
                                    aldp[:, k, :], Abuf[:, j, :], ald16[:, b, :],
                                    start=(firsts[k] == b0 + j),
                                    stop=(lasts[k] == b0 + j))
                        u = gp.tile([128, n_ch * HEADS], fp32, tag="gu", name="gu",
                                    bufs=3)
                        nc.vector.tensor_tensor(
                            u[:].rearrange("p (k h) -> p k h", h=HEADS),
                            m[:, :, HID:HID + 8].bitcast(fp32),
                            aldp[:, 0:n_ch, :], op=ALU.add)
                        nc.vector.memset(m[:, :, HID:HID + 8], 0.0)
                        # leaky_relu as max(u, 0.2u) on DVE: avoids thrashing
                        # the activation-function table between Lrelu and Exp
                        lr = gp.tile([128, n_ch * HEADS], fp32, tag="glr", name="glr",
                                     bufs=3)
                        nc.vector.scalar_tensor_tensor(
                            lr[:], u[:], NEG, u[:], op0=ALU.mult, op1=ALU.max)
                        lsh = gp.tile([128, n_ch * HEADS], fp32, tag="glsh", name="glsh",
                                      bufs=3)
                        nc.vector.tensor_tensor(
                            lsh[:].rearrange("p (k h) -> p k h", h=HEADS),
                            lr[:].rearrange("p (k h) -> p k h", h=HEADS),
                            crep[:].unsqueeze(1).broadcast_to([128, n_ch, HEADS]),
                            op=ALU.subtract)
                        # exp weights duplicated in pairs so the hg-weighting
                        # multiply keeps packed innermost dims (DVE 2x mode)
                        expe = gp.tile([128, n_ch, HEADS, 2], fp16, tag="gex", name="gex",
                                       bufs=3)
                        nc.scalar.activation(
                            expe[:, :, :, 0], lsh[:].rearrange("p (k h) -> p k h", h=HEADS),
                            ACT.Exp)
                        nc.scalar.activation(
                            expe[:, :, :, 1], lsh[:].rearrange("p (k h) -> p k h", h=HEADS),
                            ACT.Exp)
                        # write exp weights as message cols + weight hg in place
                        nc.vector.tensor_copy(
                            m[:, :, HID + 8:HID + 12], expe[:, :, :, 0])
                        nc.vector.tensor_tensor(
                            m[:, :, 0:HID].rearrange("p k (h a b) -> p k h a b", a=32, b=2),
                            m[:, :, 0:HID].rearrange("p k (h a b) -> p k h a b", a=32, b=2),
                            expe[:].unsqueeze(3).broadcast_to([128, n_ch, HEADS, 32, 2]),
                            op=ALU.mult)
                        for uu in range(un):
                            kg, b = use_flat[us + uu]
                            done[b] += 1
                            nc.tensor.matmul(
                                paggs[b][:], S[:, uu, :], m[:, kg - start, 0:DM],
                                start=False, stop=(done[b] == int(total_uses[b])))

                    def gat_evict_group(g):
                        dens, rdens, t2s, hatts = {}, {}, {}, {}
                        for b in g:
                            den = gp.tile([128, HEADS], fp32, tag="gden", name="gden", bufs=5)
                            nc.scalar.copy(den[:], paggs[b][:, HID + 8:HID + 12])
                            dens[b] = den
                        for b in g:
                            nc.vector.tensor_scalar(dens[b][:], dens[b][:], 1e-30, None, op0=ALU.max)
                            rden = gp.tile([128, HEADS], fp32, tag="grden", name="grden", bufs=5)
                            nc.vector.reciprocal(rden[:], dens[b][:])
                            rdens[b] = rden
                        for b in g:
                            pg = paggs.pop(b)
                            t1 = gp.tile([128, HID], fp32, tag="gt1", name="gt1", bufs=5)
                            nc.vector.tensor_tensor(
                                t1[:].rearrange("p (h f) -> p h f", h=HEADS),
                                pg[:, 0:HID].rearrange("p (h f) -> p h f", h=HEADS),
                                rdens[b][:].unsqueeze(2).broadcast_to([128, HEADS, FH]),
                                op=ALU.mult)
                            t2 = gp.tile([128, HID], fp32, tag="gt2", name="gt2", bufs=5)
                            nc.vector.tensor_tensor(t2[:], t1[:], bgr[:], op=ALU.add)
                            t2s[b] = t2
                        for b in g:
                            r = rows(b)
                            hatt = gp.tile([128, HID], fp16, tag="ghat", name="ghat", bufs=5)
                            if r < 128:
                                # relu(t2*mask) == mask*relu(t2) for mask in {0,1}
                                nc.scalar.activation(hatt[:], t2s[b][:], ACT.Relu, scale=rowmask[:])
                            else:
                                nc.scalar.activation(hatt[:], t2s[b][:], ACT.Relu)
                            hatts[b] = hatt
                        for b in g:
                            nc.tensor.matmul(ppool0[:], hatts[b][:, 0:128], ones_c[:],
                                             start=(b == 0), stop=(b == NB - 1))
                            nc.tensor.matmul(ppool1[:], hatts[b][:, 128:256], ones_c[:],
                                             start=(b == 0), stop=(b == NB - 1))

                    for (lo_s, lo_n, hi_s, hi_n, g, lo_us, lo_un,
                         hi_us, hi_un) in layout:
                        gn = len(g)
                        g0 = g[0]
                        # local rows for the self-loop term
                        hgl = gp.tile([128, GBLK, GSLOT], fp16, tag="ghgl", name="hgl")
                        if g[-1] == NB - 1:
                            nc.vector.memset(hgl[:], 0.0)
                            full = gn - 1
                            if full:
                                nc.sync.dma_start(
                                    hgl[:, 0:full, :],
                                    gtloc[g0 * 128:(g0 + full) * 128, :]
                                    .rearrange("(g p) d -> p g d", p=128))
                            nc.sync.dma_start(
                                hgl[:LASTB, full, :],
                                gtloc[(NB - 1) * 128:NPC, :])
                        else:
                            nc.sync.dma_start(
                                hgl[:, 0:gn, :],
                                gtloc[g0 * 128:(g0 + gn) * 128, :]
                                .rearrange("(g p) d -> p g d", p=128))
                        rhs = gp.tile([128, GBLK, DM], fp16, tag="grhs", name="grhs")
                        nc.vector.tensor_tensor(
                            rhs[:, 0:gn, 0:HID].rearrange(
                                "p g (h a b) -> p g h a b", a=32, b=2),
                            hgl[:, 0:gn, 0:HID].rearrange(
                                "p g (h a b) -> p g h a b", a=32, b=2),
                            wself[:, g0:g0 + gn].unsqueeze(3).broadcast_to(
                                [128, gn, HEADS, 32, 2]),
                            op=ALU.mult)
                        nc.vector.memset(rhs[:, 0:gn, HID:HID + 8], 0.0)
                        nc.vector.tensor_copy(
                            rhs[:, 0:gn, HID + 8:HID + 12], wself[:, g0:g0 + gn, :, 0])
                        for b_i, b in enumerate(g):
                            paggs[b] = gpp.tile([128, DM], fp32, tag="gagg", name="gagg")
                            nc.tensor.matmul(
                                paggs[b][:], ident[:], rhs[:, b_i, :],
                                start=True, stop=(int(total_uses[b]) == 0))
                        gat_span(lo_s, lo_n, lo_us, lo_un, tab_lo)
                        gat_span(hi_s, hi_n, hi_us, hi_un, tab_hi)
                        gat_evict_group(g)

                # ---------- pooling + AllReduce + MLP ----------
                with (tc.tile_pool(name=f"mlp_sb_{rep}", bufs=1) as mp,
                      tc.tile_pool(name=f"mlp_ps_{rep}", bufs=1, space="PSUM") as mpp):
                    pool_sb = mp.tile([128, 2], fp32, name="pool_sb")
                    nc.scalar.copy(pool_sb[:, 0:1], ppool0[:])
                    nc.scalar.copy(pool_sb[:, 1:2], ppool1[:])
                    # (ppool0/ppool1 are separate PSUM tiles: interleaved
                    # accumulation groups within one PSUM tile lose updates)
                    nc.sync.dma_start(arin[:], pool_sb[:])
                    maybe_cc("AllReduce", ALU.add, RG, [arin[:]], [arout[:]])
                    pooled = mp.tile([128, 2], fp32, name="pooled")
                    nc.sync.dma_start(pooled[:], arout[:])
                    nc.vector.tensor_scalar(pooled[:], pooled[:], 1.0 / N, None, op0=ALU.mult)
                    pz1 = mpp.tile([128, 1], fp32, tag="pz", name="pz1")
                    nc.tensor.matmul(pz1[:], wc1[:, 0:128], pooled[:, 0:1], start=True, stop=False)
                    nc.tensor.matmul(pz1[:], wc1[:, 128:256], pooled[:, 1:2], start=False, stop=True)
                    z1 = mp.tile([128, 1], fp32, name="z1")
                    nc.scalar.activation(z1[:], pz1[:], ACT.Relu, bias=bc1[:])
                    pz2 = mpp.tile([64, 1], fp32, tag="pz", name="pz2")
                    nc.tensor.matmul(pz2[:], wc2[:], z1[:], start=True, stop=True)
                    z2 = mp.tile([64, 1], fp32, name="z2")
                    nc.scalar.activation(z2[:], pz2[:], ACT.Relu, bias=bc2[:])
                    pz3 = mpp.tile([8, 1], fp32, tag="pz", name="pz3")
                    nc.tensor.matmul(pz3[:], wc3[:], z2[:64, :], start=True, stop=True)
                    zo = mp.tile([8, 1], fp32, name="zo")
                    nc.scalar.activation(zo[:], pz3[:], ACT.Identity, bias=bc3[:])
                    nc.sync.dma_start(out_d[:], zo[:])
                plp_cm.__exit__(None, None, None)

            for _rep in range(repeat):
                run_body(_rep)


    nc.compile()
    return nc


# --------------------------------------------------------------------------
# entry point
# --------------------------------------------------------------------------

def kernel(**inputs):
    x = np.asarray(inputs["x"], dtype=np.float32)
    ei = np.asarray(inputs["edge_index"], dtype=np.int64)
    sched = _preprocess(x, ei)
    nc = _build(sched)

    W = {k: np.asarray(v, dtype=np.float32) for k, v in inputs.items()
         if k not in ("x", "edge_index")}

    def pack_k(w, nslab):   # [K, M] -> [128, nslab*M] (row-slab packed)
        K, M = w.shape
        out = np.zeros((128, nslab * M), np.float32)
        for s in range(nslab):
            r0 = s * 128
            r1 = min(K, r0 + 128)
            out[0:r1 - r0, s * M:(s + 1) * M] = w[r0:r1]
        return out


    common = {
        "w1_d": pack_k(W["W1"], 1).astype(F16),
        "w2_d": pack_k(W["W2"], 2).astype(F16),
        "w3_d": pack_k(W["W3"], 2).astype(F16),
        "wg_d": np.concatenate([
            W["Wg"],
            (W["Wg"].reshape(128, HEADS, FH) * W["a_src"][None]).sum(-1),
            (W["Wg"].reshape(128, HEADS, FH) * W["a_dst"][None]).sum(-1)],
            axis=1).astype(F16),
        "b1_d": W["b1"].reshape(1, -1).astype(F16),
        "b2_d": W["b2"].reshape(1, -1).astype(F16),
        "b3_d": np.tile(W["b3"].reshape(1, -1), (128, 1)).astype(np.float32),
        "bg_d": np.tile(W["bg"].reshape(1, -1), (128, 1)).astype(np.float32),
        "wc1_d": pack_k(W["Wc1"], 2).astype(np.float32),
        "wc2_d": pack_k(W["Wc2"], 1)[:, :64].astype(np.float32),
        "wc3_d": pack_k(W["Wc3"], 1)[:64, :8].astype(np.float32),
        "bc1_d": W["bc1"].reshape(-1, 1).astype(np.float32),
        "bc2_d": W["bc2"].reshape(-1, 1).astype(np.float32),
        "bc3_d": W["bc3"].reshape(-1, 1).astype(np.float32),
        "rowmask_d": (np.arange(128) < LASTB).astype(np.float32).reshape(128, 1),
    }

    NUSE = sched["NUSE"]
    in_maps = []
    for c in range(NCORES):
        rel = sched["dstrel"][c]
        in_maps.append(dict(
            common,
            xs=np.ascontiguousarray(x[c * NPC:(c + 1) * NPC]),
            idxs_d=sched["idxs"][c],
            dstrel_d=np.ascontiguousarray(
                np.repeat(rel.reshape(NUSE, 128).T, 2, axis=1)).astype(F16),
            dinv_d=sched["dinv"][c],
        ))

    res = run_bass_kernel_spmd(nc, in_maps, core_ids=list(range(NCORES)))
    global LAST_RESULT
    LAST_RESULT = res
    return res.results[0]["out_d"].reshape(1, OUT).astype(np.float32)


LAST_RESULT = None


